# revision 103
# baseline (speedup 1.0000x reference)
"""Trainium2 Bass kernel: Wan-style interleaved RoPE on q/k + causal attention.

Full problem: q,k,v [B=2, S=2048, H=16, D=128] fp32, freqs [1, S, 1, D].
  rq = rope(q), rk = rope(k)
  out[b,h,q,d] = softmax_causal(rq @ rk^T / sqrt(D)) @ v      -> [B, H, S, D]

Sharding: heads split across 8 cores (2 heads/core); each core handles
4 independent (b, h) attention problems. Inputs are sliced on host, the
SPMD kernel runs on cores 0-7, outputs are concatenated on host.

Layout trick: scores = sum_d rq[d]*rk[d] are invariant under any shared
permutation of d, so q and k are shipped de-interleaved (evens then
odds) AND pre-transposed to [D', S] on the host.  The vector engines
have no cross-partition path, so the even/odd halves are loaded
DUPLICATED (two half-DMAs from the same DRAM rows):
  qE = (x0|x0), qO = (x1|x1), FF = (f0|f1), GG = (-f1|f0)
  rqT' = qE*FF + qO*GG    -- exactly interleaved RoPE in (evens|odds)
                             d-order.

Everything is shipped and computed in fp16 (rel err ~4e-3 vs the fp32
reference): fp16 moving operands stream the PE at 1 cyc/row at EVERY
free size (f32r degrades to 4 cyc/row below 256 cols), DMA -- which is
packet-processing-bound, not bandwidth-bound -- moves half the packets,
and the 16-bit RoPE ops get 2x DVE throughput.

Attention per (b,h), per q-block of 512 columns: k-tiles are bin-packed
into 512-col PSUM banks (a matmul may not cross a bank boundary), three
banks per [128, 1536] score tile, diagonal tiles reordered j0,j1,j3,j2
so the packing has no holes; one exp instruction covers each batch.
Softmax uses exp(s*scale - 8): the uniform bias cancels in the
normalization and keeps exp outputs inside fp16 range (max scaled score
~15.4).  probT is fp16; PV streams it at 1 cyc/row.

Softmax sums: probT tiles are accumulated on DVE (fp16 adds; entries
bounded by nk*exp(15.4-8) ~ 27k < fp16 max) and ONE 512-col ones-matmul
per q-block reduces the accumulator across partitions.  The per-tile
M=1 sums matmuls they replace streamed as many PE columns as PV itself
(~45us at the power-capped ~0.73ns/col PE rate).

Per q-block normalize (deferred past the next q-block's score matmuls,
PIPE, so the PE never waits on DVE/ACT round trips): sums are
PE-transposed (4 tiny 1-col transposes), reciprocated on DVE, the PV
accumulator is evacuated fp32->fp16 by ScalarE (its queue holds only
exps, so this never queues behind RoPE on DVE), PE-transposed back to
[q, d] and scaled on DVE into a [128, S] fp16 buffer that is DMA'd once
per (b,h) (4KB rows) and upcast to fp32 on host.

Boot: (b,h)=0's q/k are shipped a second time chunk-tiled [4, D, 512]
(each 512-col chunk contiguous in DRAM) so the boot loads+RoPE run
chunk-by-chunk and qb0's matmuls start early; dma_start issue cost
(~650ns each on a sequencer queue) is spread over the Sync, GpSimd and
Scalar queues.  In steady state the next (b,h)'s load DMAs issue at the
start of the current attention and its RoPE is emitted after q-block 1,
so every engine queue stays fed (queues drain in emission order).

Measured (min of 4): ~150.0us on hardware, rel err 4.1e-3 (vs 186-193us
for the f32r baseline).  probT pool is 6-deep so ACT's exps run well
ahead of the PV matmuls at batch boundaries.  Engine busy: DVE ~110us (bottleneck: RoPE +
accumulate adds + normalize), PE ~95us, ScalarE ~87us (exp is
irreducible at ~0.83ns/col), DMA ~65us.  Measured dead ends: moving DVE
work to GpSimd (Pool tensor_tensor is ~3x slower and lands on the
(b,h)-transition critical path), strided tensor_reduce for non-first
batches (slower than plain adds on HW), fp8 probs (softmax dynamic
range), and PE/GpSimd broadcast normalizes (FIFO ping-pong / microcode
overhead).
"""

import math

import numpy as np

B, S, H, D = 2, 2048, 16, 128
NCORES = 8
HPC = H // NCORES          # heads per core
NBH = B * HPC              # (b, h) problems per core
NT = S // 128              # s-tiles
QB = S // 512              # q blocks of 512
SCALE = 1.0 / math.sqrt(D)
NEG = -1e30
EXPBIAS = 8.0              # uniform softmax shift; keeps exp in fp16 range
SCW = 1536                 # packed score-tile width (3 PSUM banks)
PIPE = True                # defer q-block i's normalize past i+1's scores
HOOK_QB = 1                # q-block after which the next (b,h)'s RoPE is emitted

_CACHE = {}


def _plan(qb):
    """Pack this q-block's k-tiles into contiguous score batches.

    A matmul output must not cross a 512-col PSUM bank boundary, so tiles
    are bin-packed into 512-col banks (3 banks per [128, SCW] score
    tile).  The diagonal tiles (widths 512/384/256/128) are emitted in
    the order j0, j1, j3, j2 so banks fill exactly ([512], [384+128],
    [256]) with no holes: each batch's valid columns are contiguous from
    0 and one exp instruction covers them.  The first tile (tk=0, full
    width) stays first so its start=True matmul resets every PSUM cell
    of the PV/sums accumulators.

    Returns (nk, batches); each batch is a list of (tk, off, lo, w).
    """
    nk = 4 * qb + 4
    order = list(range(4 * qb)) + [4 * qb, 4 * qb + 1, 4 * qb + 3, 4 * qb + 2]
    batches, cur = [], []
    bank, used = 0, 0
    for tk in order:
        j = tk - 4 * qb
        off = 128 * j if j > 0 else 0
        w = 512 - off
        if used + w > 512:
            bank, used = bank + 1, 0
        if bank == SCW // 512:
            batches.append(cur)
            cur, bank = [], 0
        cur.append((tk, off, bank * 512 + used, w))
        used += w
    batches.append(cur)
    return nk, batches


def _build():
    import concourse.mybir as mybir
    import concourse.tile as tile
    from concourse import bacc
    from concourse.masks import make_identity

    f32 = mybir.dt.float32
    f16 = mybir.dt.float16
    bf16 = mybir.dt.bfloat16
    Alu = mybir.AluOpType
    Act = mybir.ActivationFunctionType

    nc = bacc.Bacc("TRN2", target_bir_lowering=False, debug=False,
                   num_devices=NCORES)
    qd = nc.dram_tensor("qT", [NBH, D, S], f16, kind="ExternalInput")
    kd = nc.dram_tensor("kT", [NBH, D, S], f16, kind="ExternalInput")
    vd = nc.dram_tensor("v", [NBH, 128, S], f16, kind="ExternalInput")
    # boot copies of (b,h)=0's q/k, PRE-DUPLICATED on host ((E|E) and
    # (O|O) half layouts) and chunk-tiled [4, 128, 512] (each chunk one
    # contiguous DRAM block): boot loads+RoPE run chunk-by-chunk so
    # qb0's matmuls start early, with ONE dma_start per tile instead of
    # two half-loads (boot is serialized by the ~650ns issue cost).
    qbe = nc.dram_tensor("qTbE", [4, 128, 512], f16, kind="ExternalInput")
    qbo = nc.dram_tensor("qTbO", [4, 128, 512], f16, kind="ExternalInput")
    kbe = nc.dram_tensor("kTbE", [4, 128, 512], f16, kind="ExternalInput")
    kbo = nc.dram_tensor("kTbO", [4, 128, 512], f16, kind="ExternalInput")
    fd = nc.dram_tensor("freqsT", [4, D, 512], f16, kind="ExternalInput")
    gd = nc.dram_tensor("freqsG", [4, D, 512], f16, kind="ExternalInput")
    od = nc.dram_tensor("out", [NBH, 128, S], f16, kind="ExternalOutput")

    with tile.TileContext(nc) as tc:
        with (
            tc.tile_pool(name="const", bufs=1) as cpool,
            tc.tile_pool(name="io", bufs=2) as iopool,
            tc.tile_pool(name="rope", bufs=2) as rpool,
            tc.tile_pool(name="xt", bufs=2) as xtpool,
            tc.tile_pool(name="prob", bufs=8) as ppool,
            tc.tile_pool(name="small", bufs=2) as spool,
            tc.tile_pool(name="outf", bufs=2) as opool,
            tc.tile_pool(name="sc_ps", bufs=2, space="PSUM") as sc_ps,
            tc.tile_pool(name="out_ps", bufs=1, space="PSUM") as out_ps,
            tc.tile_pool(name="mp_ps", bufs=1, space="PSUM") as mp_ps,
        ):
            # ---- constants ----
            ident = cpool.tile([128, 128], f32, tag="ident")
            make_identity(nc, ident[:])
            # tri_bf[k, t] = 0 where k <= t (valid), NEG where k > t.
            tri_bf = cpool.tile([128, 128], bf16, tag="tri_bf")
            nc.gpsimd.memset(tri_bf[:], 0.0)
            nc.gpsimd.affine_select(
                out=tri_bf[:], in_=tri_bf[:],
                compare_op=Alu.is_ge, fill=NEG, base=0,
                pattern=[[1, 128]], channel_multiplier=-1,
            )
            ident_bf = cpool.tile([128, 128], bf16, tag="ident_bf")
            nc.vector.tensor_copy(ident_bf[:], ident[:])
            ones_f32 = cpool.tile([128, 1], f32, tag="ones_f32")
            nc.vector.memset(ones_f32[:], 1.0)
            ones_col = cpool.tile([128, 1], f16, tag="ones")
            nc.vector.tensor_copy(ones_col[:], ones_f32[:])
            ident_f16 = cpool.tile([128, 128], f16, tag="ident_f16")
            nc.vector.tensor_copy(ident_f16[:], ident[:])
            nbias = cpool.tile([128, 1], f32, tag="nbias")
            nc.vector.memset(nbias[:], -EXPBIAS)
            FF = cpool.tile([128, S], f16, tag="FF")
            GG = cpool.tile([128, S], f16, tag="GG")

            def rope_dma(bh, xd, te, to, eq, oq):
                # eq/oq: issue queues for the E/O half-DMAs; spreading
                # across sequencers parallelizes the ~650ns issue cost.
                xE = rpool.tile([128, S], f16, tag=te, name=te)
                xO = rpool.tile([128, S], f16, tag=to, name=to)
                eq.dma_start(xE[0:64, :], xd.ap()[bh, 0:64, :])
                eq.dma_start(xE[64:128, :], xd.ap()[bh, 0:64, :])
                oq.dma_start(xO[0:64, :], xd.ap()[bh, 64:128, :])
                oq.dma_start(xO[64:128, :], xd.ap()[bh, 64:128, :])
                return xE, xO

            def rope_compute(xE, xO, xT_ap, mulo_eng, cs=slice(0, S)):
                nc.vector.tensor_mul(xE[:], xE[:], FF[:, cs])
                mulo_eng.tensor_mul(xO[:], xO[:], GG[:, cs])
                nc.vector.tensor_add(xT_ap, xE[:], xO[:])

            def emit_load_boot():
                """Chunked load+RoPE for (b,h)=0 from the chunk-tiled boot
                tensors: qb0's matmuls start as soon as chunk 0 lands.
                E-half muls on DVE, O-half on GpSimd (they run in
                parallel); freqs/v issue from the idle Scalar queue."""
                qTc = [xtpool.tile([128, 512], f16, tag=f"bqT{c}",
                                   name=f"bqT{c}", bufs=1) for c in range(4)]
                kTc = [xtpool.tile([128, 512], f16, tag=f"bkT{c}",
                                   name=f"bkT{c}", bufs=1) for c in range(4)]
                for c in range(4):
                    cs = slice(c * 512, (c + 1) * 512)
                    nc.scalar.dma_start(FF[:, cs], fd.ap()[c])
                    nc.scalar.dma_start(GG[:, cs], gd.ap()[c])
                    kE = rpool.tile([128, 512], f16, tag="kE", name="kE")
                    kO = rpool.tile([128, 512], f16, tag="kO", name="kO")
                    nc.sync.dma_start(kE[:], kbe.ap()[c])
                    nc.gpsimd.dma_start(kO[:], kbo.ap()[c])
                    rope_compute(kE, kO, kTc[c][:], nc.gpsimd, cs)
                    qE = rpool.tile([128, 512], f16, tag="qE", name="qE")
                    qO = rpool.tile([128, 512], f16, tag="qO", name="qO")
                    nc.sync.dma_start(qE[:], qbe.ap()[c])
                    nc.gpsimd.dma_start(qO[:], qbo.ap()[c])
                    rope_compute(qE, qO, qTc[c][:], nc.gpsimd, cs)
                    if c == 0:
                        v_mm = iopool.tile([128, S], f16, tag="v_mm",
                                           name="v_mm")
                        nc.scalar.dma_start(v_mm[:], vd.ap()[0])

                def kT_lhsT(tk):
                    return kTc[tk // 4][:, (tk % 4) * 128:(tk % 4 + 1) * 128]

                def qT_rhs(qb, off):
                    return qTc[qb][:, off:512]

                return (kT_lhsT, qT_rhs, v_mm)

            def emit_load_dma(bh):
                # prefetch DMAs only -- emitted early so transfers overlap
                # the previous attention without head-of-line blocking the
                # vector queues.
                v_mm = iopool.tile([128, S], f16, tag="v_mm", name="v_mm")
                nc.gpsimd.dma_start(v_mm[:], vd.ap()[bh])
                qT = xtpool.tile([128, S], f16, tag="qT", name="qT")
                kT = xtpool.tile([128, S], f16, tag="kT", name="kT")
                kEO = rope_dma(bh, kd, "kE", "kO", nc.sync, nc.gpsimd)
                qEO = rope_dma(bh, qd, "qE", "qO", nc.sync, nc.gpsimd)

                def kT_lhsT(tk):
                    return kT[:, tk * 128:(tk + 1) * 128]

                def qT_rhs(qb, off):
                    return qT[:, qb * 512 + off:(qb + 1) * 512]

                return (kT_lhsT, qT_rhs, v_mm), (kEO, kT, qEO, qT)

            def emit_load_compute(parts):
                # k's O-half mul goes to GpSimd (it has until the end of
                # the current attention); q's stays on DVE.
                (kE, kO), kT, (qE, qO), qT = parts
                rope_compute(kE, kO, kT[:], nc.gpsimd)
                rope_compute(qE, qO, qT[:], nc.vector)

            def emit_attention(bh, acc, dma_hook, compute_hook):
                kT_lhsT, qT_rhs, v_mm = acc
                last = bh == NBH - 1
                if dma_hook is not None:
                    dma_hook()
                out_full = opool.tile([128, S], f16, tag="out_full",
                                      name="out_full")
                store = {}

                def phase_compute(qb):
                    nk, batches = _plan(qb)
                    last_tk = batches[-1][-1][0]
                    outT = out_ps.tile([128, 512], f32, tag="outT",
                                       name="outT")
                    # probT tiles are also accumulated on DVE (fp16 adds,
                    # entries bounded by nk*exp(15.4-EXPBIAS) ~ 27k, under
                    # fp16 max): ONE 512-col ones-matmul per q-block then
                    # replaces the 40 per-tile sums matmuls per (b,h) --
                    # at the power-capped ~0.73ns/col PE rate those cost
                    # more than the PV matmuls themselves.
                    acc = spool.tile([128, 512], f16, tag="acc", name="acc")
                    first = True
                    for batch in batches:
                        sc = sc_ps.tile([128, SCW], f32, tag="sc", name="sc")
                        for tk, off, lo, w in batch:
                            diag = tk >= 4 * qb
                            nc.tensor.matmul(
                                sc[:, lo:lo + w], kT_lhsT(tk),
                                qT_rhs(qb, off),
                                start=True, stop=not diag,
                            )
                            if diag:
                                nc.tensor.matmul(
                                    sc[:, lo:lo + 128],
                                    ident_bf[:], tri_bf[:],
                                    start=False, stop=True,
                                )
                        wtot = batch[-1][2] + batch[-1][3]
                        probt = ppool.tile([128, SCW], f16, tag="probt",
                                           name="probt")
                        nc.scalar.activation(
                            probt[:, 0:wtot], sc[:, 0:wtot],
                            Act.Exp, scale=SCALE, bias=nbias[:],
                        )
                        for tk, off, lo, w in batch:
                            nc.tensor.matmul(
                                outT[:, off:512],
                                v_mm[:, tk * 128:(tk + 1) * 128],
                                probt[:, lo:lo + w],
                                start=(tk == 0), stop=(tk == last_tk),
                            )
                        if first and len(batch) == 3 and \
                                all(t[1] == 0 for t in batch):
                            # full 3-tile first batch: one strided
                            # tensor_reduce sums all three 512-col tiles
                            # straight into acc (replaces copy + 2 adds).
                            with nc.allow_low_precision("fp16 psums"):
                                nc.vector.tensor_reduce(
                                    acc[:],
                                    probt[:, 0:1536].rearrange(
                                        "p (t q) -> p q t", t=3),
                                    mybir.AxisListType.X, Alu.add,
                                )
                            first = False
                            continue
                        for tk, off, lo, w in batch:
                            if first:
                                nc.vector.tensor_copy(
                                    acc[:], probt[:, lo:lo + w])
                                first = False
                            else:
                                nc.vector.tensor_add(
                                    acc[:, off:512], acc[:, off:512],
                                    probt[:, lo:lo + w])
                    # free outT promptly on ScalarE (GpSimd cannot touch
                    # PSUM; ScalarE's queue holds only exps, so this never
                    # queues behind the next (b,h)'s RoPE on DVE).
                    outT_sb = spool.tile([128, 512], f16, tag="outT_sb",
                                         name="outT_sb")
                    nc.scalar.copy(outT_sb[:], outT[:])
                    store[qb] = (acc, outT_sb)

                def phase_store(qb):
                    # with PIPE on, this is emitted after q-block qb+1's
                    # compute: the ones-matmul and sT transposes queue
                    # behind a long matmul stretch, so the DVE adds they
                    # depend on are long since done and the PE never waits.
                    acc, outT_sb = store.pop(qb)
                    sums = mp_ps.tile([1, 512], f32, tag="mp", name="sums")
                    nc.tensor.matmul(sums[:], ones_col[:], acc[:],
                                     start=True, stop=True)
                    sums_sb = spool.tile([1, 512], f32, tag="sums_sb",
                                         name="sums_sb")
                    nc.vector.tensor_copy(sums_sb[:], sums[:])
                    sT = mp_ps.tile([128, 4], f32, tag="mp", name="sT")
                    for j in range(4):
                        nc.tensor.transpose(
                            sT[:, j:j + 1],
                            sums_sb[0:1, j * 128:(j + 1) * 128],
                            ident[0:1, 0:1],
                        )
                    recip = spool.tile([128, 4], f32, tag="recip",
                                       name="recip")
                    nc.vector.reciprocal(recip[:], sT[:])
                    o_ps = mp_ps.tile([128, 512], f16, tag="mp",
                                      name="o_ps")
                    for j in range(4):
                        nc.tensor.transpose(
                            o_ps[:, j * 128:(j + 1) * 128],
                            outT_sb[:, j * 128:(j + 1) * 128],
                            ident_f16[:],
                        )
                    # ONE fused mul for all four 128-col chunks: recip
                    # [128, 4] is broadcast along a stride-0 inner dim to
                    # [128, 4, 128] so each chunk sees its own scalar.
                    nc.vector.tensor_mul(
                        out_full[:, qb * 512:(qb + 1) * 512]
                        .rearrange("p (f r) -> p f r", f=4),
                        o_ps[:].rearrange("p (f r) -> p f r", f=4),
                        recip[:].unsqueeze(2).broadcast_to([128, 4, 128]),
                    )
                    if last:
                        # no next (b,h) overlaps the tail: flush each
                        # q-block as soon as it is normalized.
                        nc.sync.dma_start(
                            od.ap()[bh, :, qb * 512:(qb + 1) * 512],
                            out_full[:, qb * 512:(qb + 1) * 512])

                if PIPE:
                    for qb in range(QB):
                        phase_compute(qb)
                        if qb >= 1:
                            phase_store(qb - 1)
                        if qb == HOOK_QB and compute_hook is not None:
                            compute_hook()
                    phase_store(QB - 1)
                else:
                    for qb in range(QB):
                        phase_compute(qb)
                        phase_store(qb)
                        if qb == HOOK_QB and compute_hook is not None:
                            compute_hook()
                if not last:
                    nc.sync.dma_start(od.ap()[bh], out_full[:])

            accs = {0: emit_load_boot()}
            parts = {}

            for bh in range(NBH):
                def dma_hook(bh=bh):
                    if bh + 1 < NBH:
                        accs[bh + 1], parts[bh + 1] = emit_load_dma(bh + 1)

                def compute_hook(bh=bh):
                    if bh + 1 < NBH:
                        emit_load_compute(parts[bh + 1])
                emit_attention(bh, accs[bh], dma_hook, compute_hook)

    nc.compile()
    return nc


def _get_nc():
    if "nc" not in _CACHE:
        _CACHE["nc"] = _build()
    return _CACHE["nc"]


def _deint_T(x):
    # [N, S, D] -> de-interleave d (evens|odds) then transpose -> [N, D, S]
    return np.ascontiguousarray(
        np.concatenate([x[:, :, 0::2], x[:, :, 1::2]], axis=2)
        .transpose(0, 2, 1)).astype(np.float16)


def _shard(q, k, v, freqs):
    q = np.asarray(q, dtype=np.float32)
    k = np.asarray(k, dtype=np.float32)
    v = np.asarray(v, dtype=np.float32)
    freqs = np.asarray(freqs, dtype=np.float32).reshape(S, D)
    def _ctile(t):
        # [D, S] fp16 -> chunk-tiled [4, D, 512] (each chunk contiguous)
        return np.ascontiguousarray(t.reshape(D, 4, 512).transpose(1, 0, 2))

    fT = _ctile(np.concatenate([freqs[:, 0::2], freqs[:, 1::2]], axis=1)
                .T.astype(np.float16))
    gT = _ctile(np.concatenate([-freqs[:, 1::2], freqs[:, 0::2]], axis=1)
                .T.astype(np.float16))
    in_maps = []
    for c in range(NCORES):
        h0 = c * HPC

        def bhslice(x):
            # [B, S, Hc, D] -> [B, Hc, S, D] -> [NBH, S, D]
            return np.ascontiguousarray(
                x[:, :, h0:h0 + HPC, :].transpose(0, 2, 1, 3)
            ).reshape(NBH, S, D)

        # v s-tiled: vt[bh, p, t*128+d] = v[bh, t*128+p, d]
        vt = np.ascontiguousarray(
            bhslice(v).reshape(NBH, NT, 128, D).transpose(0, 2, 1, 3)
        ).reshape(NBH, 128, S).astype(np.float16)

        qT = _deint_T(bhslice(q))
        kT = _deint_T(bhslice(k))
        in_maps.append({
            "qT": qT,
            "kT": kT,
            "qTbE": _ctile(np.concatenate([qT[0][0:64], qT[0][0:64]])),
            "qTbO": _ctile(np.concatenate([qT[0][64:128], qT[0][64:128]])),
            "kTbE": _ctile(np.concatenate([kT[0][0:64], kT[0][0:64]])),
            "kTbO": _ctile(np.concatenate([kT[0][64:128], kT[0][64:128]])),
            "v": vt,
            "freqsT": fT,
            "freqsG": gT,
        })
    return in_maps


def kernel(q, k, v, freqs):
    nc = _get_nc()
    from concourse.bass_utils import run_bass_kernel_spmd

    in_maps = _shard(q, k, v, freqs)
    res = run_bass_kernel_spmd(nc, in_maps, core_ids=list(range(NCORES)))

    out = np.empty((B, H, S, D), dtype=np.float32)
    for c in range(NCORES):
        h0 = c * HPC
        # out dram [NBH, 128, S]: [bh, p, qb*512 + j*128 + d] is output
        # row qb*512 + j*128 + p, col d.
        r = res.results[c]["out"].astype(np.float32)
        r = r.reshape(B, HPC, 128, QB, 4, D)
        r = r.transpose(0, 1, 3, 4, 2, 5).reshape(B, HPC, S, D)
        out[:, h0:h0 + HPC] = r
    return out
